# revision 36
# baseline (speedup 1.0000x reference)
"""BDGCN (dual-diffusion graph conv) Trainium2 kernel.

Math (per batch b):
  m1[k,m,c,l] = sum_n X[n,c,l] G[k,n,m]
  m2[m,d,k,j,l] = sum_c m1[k,m,c,l] G[j,c,d]
  out[m,d,h] = relu(sum_{k,j,l} m2[m,d,k,j,l] W[k*96+j*32+l, h] + b[h])

Sharding: data-parallel over batch; B=8 -> one batch per NeuronCore,
G/W/b replicated. No collectives.

Per-core pipeline (phase 1/2 bf16 operands, phase 3 m2/W float32r;
all psum accumulation fp32):
  Phase 1 (contract n): lhsT = X[n, c-chunk @ fixed l] bf16,
           rhs = G_k [n, m-full 256] bf16 -> psum [c128, (l2,m256)=512]
           batching 2 l-planes per psum bank, accum over 2 n-chunks ->
           one drain per 2 l's into M1[k][cchk] SBUF bf16, free layout
           (g64,l32,r4) where m = 4g + r.
  Phase 2 (contract c): lhsT = M1[:, 128-col block g] (cols = (l,r)),
           rhs = G_{j0|j1} [c128, (j,d)] (the two j's sharing a k reuse
           the stationary lhsT in one 512-row matmul) -> psum
           [(l,r)128, d], accum over 2 c-chunks -> M2 SBUF f32r.
  Phase 3 (contract (k,j,l)): lhsT = M2[kj][:, d-chunk],
           rhs = block-diagonal W [(l,r)128, (r,h)256] f32r -> psum
           [d128, (m4,h)256] (both d-chunks share one bank), accum over
           9 (k,j) (+1 rank-1 matmul adding bias, only when b != 0).
           relu fused with the PSUM drain on the Activation engine;
           one out-DMA per m-group covers both d-chunks.
PSUM->SBUF drains are greedily load-balanced between DVE and
Activation (GPSIMD cannot read PSUM on TRN2); PE matmul streaming is
the bottleneck at ~90% occupancy.

Walrus-build workarounds baked in: Tile's exit drain is split into
single-wait drains (_patch_tile_drain) and any instruction carrying >1
semaphore wait gets extra waits hoisted onto NoOps (_split_multi_waits).
"""

import numpy as np

B, N, L, K, H = 8, 256, 32, 3, 64
P = 128  # partitions

_CACHE = {}


def _patch_tile_drain():
    """This container's walrus build rejects instructions carrying more
    than one semaphore wait; Tile's exit emits one drain with N waits.
    Split it into N single-wait drains."""
    import concourse.mybir as mybir
    import concourse.tile as tile

    if getattr(tile.TileContext, "_drain_split_patched", False):
        return

    def patched(self, tick_clock, wait_clock):
        from concourse.vector_clock import ScopedClock

        nc = self.nc
        probe = nc.sync.drain()
        wait_clock.add_sem_waits(
            probe.ins, ScopedClock({None: tick_clock.global_clock})
        )
        si = probe.ins.sync_info
        waits = list(si.on_wait) if si is not None else []
        if len(waits) > 1:
            si.on_wait = [waits[0]]
            for w in waits[1:]:
                d = nc.sync.drain()
                d.ins.sync_info = mybir.SyncInfo(on_update=[], on_wait=[w])
        nc.all_engine_barrier()
        assert self.sems is not None
        popped = nc._tile_sem_poison_stack.pop()
        assert popped is self._sem_poison
        nc.clear_and_free_semaphores(list(self.sems.allocated().values()))
        nc.all_engine_barrier()

    tile.TileContext._drain_and_barrier = patched
    tile.TileContext._drain_split_patched = True


def _build_nc(reps=1, with_bias=False):
    import concourse.bass as bass
    import concourse.mybir as mybir
    import concourse.tile as tile
    from concourse import bacc

    _patch_tile_drain()

    f32 = mybir.dt.float32
    f32r = mybir.dt.float32r
    nc = bass.Bass("TRN2", target_bir_lowering=False, debug=False)

    bf16 = mybir.dt.bfloat16
    Xd = nc.dram_tensor("X", [N, N, L], bf16, kind="ExternalInput")
    GBd = nc.dram_tensor("GB", [K, N, N], bf16, kind="ExternalInput")
    Wr = nc.dram_tensor("WR", [K * K, P, 4 * H], f32, kind="ExternalInput")
    Bd = nc.dram_tensor("BB", [1, 4 * H], f32, kind="ExternalInput")
    Od = nc.dram_tensor("OUT", [N, N, H], f32, kind="ExternalOutput")

    NC2 = N // P  # 2 chunks of 128 along n or c
    MG = 4       # m's per group in phase 2/3
    NG = P // MG  # 32 groups per m-half
    LB = 4       # l-planes batched per phase-1 psum tile

    relu = mybir.ActivationFunctionType.Relu

    # Split PSUM->SBUF drains between DVE and Activation (GPSIMD cannot
    # read PSUM on TRN2), greedily assigning each to the engine with the
    # least accumulated cost. The fused relu drains are charged to
    # Activation as they are emitted.
    ecost = {"v": 0.0, "a": 0.0}

    def drain(i, out, in_):
        n = out.free_size()
        cv = ecost["v"] + n * 1.04 + 125
        ca = ecost["a"] + n * 0.833 + 143
        if cv <= ca:
            ecost["v"] = cv
            nc.vector.tensor_copy(out, in_)
        else:
            ecost["a"] = ca
            nc.scalar.activation(
                out, in_, mybir.ActivationFunctionType.Copy
            )

    with tile.TileContext(nc) as tc:
        with (
            tc.tile_pool(name="big", bufs=1) as big,
            tc.tile_pool(name="m2p", bufs=12) as m2p,
            tc.tile_pool(name="outp", bufs=6) as outp,
            tc.tile_pool(name="ps1", bufs=2, space="PSUM") as ps1p,
            tc.tile_pool(name="ps2", bufs=4, space="PSUM") as ps2p,
            tc.tile_pool(name="ps3", bufs=2, space="PSUM") as ps3p,
        ):
            # ---- resident loads ----
            # X split into 4 chunk DMAs so phase 1 can start on the
            # first (n-chunk, c-half) before the full load finishes
            xsb = big.tile([P, NC2 * N * L], bf16, tag="xsb")
            x4 = xsb.rearrange("p (b c l) -> p b c l", b=NC2, c=N)
            for ch in range(NC2):
                for nchk in range(NC2):
                    nc.sync.dma_start(
                        out=x4[:, nchk, ch * P:(ch + 1) * P, :],
                        in_=Xd[nchk * P:(nchk + 1) * P, ch * P:(ch + 1) * P, :],
                    )
            gbsb = big.tile([P, K * NC2 * N], bf16, tag="gbsb")
            gb4 = gbsb.rearrange("p (k b m) -> p k b m", k=K, b=NC2)
            nc.sync.dma_start(
                out=gb4,
                in_=GBd[:, :, :].rearrange("k (b p) m -> p k b m", p=P),
            )
            wsb = big.tile([P, K * K * MG * H], f32r, tag="wsb")
            w3 = wsb.rearrange("p (q c) -> p q c", q=K * K)
            nc.sync.dma_start(
                out=w3,
                in_=Wr[:, :, :].bitcast(f32r).rearrange("q p c -> p q c"),
            )
            if with_bias:
                # ones column + bias row for the rank-1 bias accumulation
                Ond = nc.dram_tensor("ONES", [1, P], f32, kind="ExternalInput")
                ones = big.tile([1, P], f32r, tag="ones")
                nc.sync.dma_start(out=ones, in_=Ond[:, :].bitcast(f32r))
                bsb = big.tile([1, MG * H], f32r, tag="bsb")
                nc.sync.dma_start(out=bsb, in_=Bd[:, :].bitcast(f32r))

            # m1 stored bf16 with the full m range per (k, c-chunk);
            # free layout (g64, l32, r4) with m = g*4 + r
            m1 = {}
            for k in range(K):
                for cc in range(NC2):
                    m1t = big.tile(
                        [P, 2 * L * P], bf16,
                        tag=f"m1_{k}_{cc}", name=f"m1_{k}_{cc}",
                    )
                    m1[k, cc] = m1t

            LB2 = 2   # l-planes per phase-1 psum bank (full-m rows)
            NGF = 2 * NG  # 64 m-groups
            ndrain = 0
            for _rep in range(reps):
                # ---- phase 1 ----
                for k in range(K):
                    for cc in range(NC2):
                        m1w = m1[k, cc].rearrange(
                            "p (g l r) -> p g l r", g=NGF, l=L
                        )
                        for lb in range(L // LB2):
                            ps = ps1p.tile([P, LB2 * N], f32, tag="ps1")
                            for li in range(LB2):
                                l = lb * LB2 + li
                                for nchk in range(NC2):
                                    nc.tensor.matmul(
                                        ps[:, li * N:(li + 1) * N],
                                        lhsT=x4[:, nchk, cc * P:(cc + 1) * P, l],
                                        rhs=gb4[:, k, nchk, :],
                                        start=(nchk == 0),
                                        stop=(nchk == NC2 - 1),
                                    )
                            # one drain per 2 l-planes, scattered into
                            # the (g, l, r) layout
                            drain(
                                ndrain,
                                m1w[:, :, lb * LB2:(lb + 1) * LB2, :],
                                ps.rearrange("p (l g r) -> p g l r", l=LB2, g=NGF),
                            )
                            ndrain += 1
                # ---- phases 2 + 3, per group of 4 m's ----
                for g in range(NGF):
                    # phase 2: for fixed k the stationary lhsT is shared
                    # across j, so j=0,1 fuse into one 512-row matmul per
                    # c-chunk (one full psum bank), j=2 runs alone.
                    m2sb = {}
                    for k in range(K):
                        for j0, j1 in ((0, 2), (2, 3)):
                            w = (j1 - j0) * N
                            ps2 = ps2p.tile([P, 2 * N], f32, tag="ps2")
                            for cc in range(NC2):
                                lv = m1[k, cc][:, g * P:(g + 1) * P]
                                nc.tensor.matmul(
                                    ps2[:, :w],
                                    lhsT=lv,
                                    rhs=gb4[:, j0:j1, cc, :],
                                    start=(cc == 0),
                                    stop=(cc == NC2 - 1),
                                )
                            t = m2p.tile([P, 2 * N], f32r, tag="m2")
                            drain(ndrain, t[:, :w], ps2[:, :w])
                            ndrain += 1
                            for j in range(j0, j1):
                                m2sb[k, j] = t[:, (j - j0) * N:(j - j0 + 1) * N]
                    # ost free layout (m4, dc2, h64) so the out-DMA's
                    # (m, dc) dims fuse into one stride-8192 dim (DMA APs
                    # are limited to 3 dims total)
                    ost = outp.tile([P, NC2 * MG * H], f32, tag="ost")
                    ost4 = ost.rearrange(
                        "p (m b h) -> p m b h", m=MG, b=NC2
                    )
                    ps3 = ps3p.tile([P, NC2 * MG * H], f32, tag="ps3")
                    for dc in range(NC2):
                        ps3v = ps3[:, dc * MG * H:(dc + 1) * MG * H]
                        for idx in range(K * K):
                            k, j = divmod(idx, K)
                            nc.tensor.matmul(
                                ps3v,
                                lhsT=m2sb[k, j][:, dc * P:(dc + 1) * P],
                                rhs=w3[:, idx, :],
                                start=(idx == 0),
                                stop=(idx == K * K - 1) and not with_bias,
                            )
                        if with_bias:
                            # rank-1 accumulation: out[d, (r,h)] += 1 * b
                            nc.tensor.matmul(
                                ps3v,
                                lhsT=ones,
                                rhs=bsb,
                                start=False,
                                stop=True,
                            )
                        # fused relu + PSUM drain on the Activation engine
                        nc.scalar.activation(ost4[:, :, dc, :], ps3v, relu)
                        ecost["a"] += MG * H * 0.833 + 143
                    mbase = g * MG
                    dst = Od[mbase:mbase + MG, :, :]
                    nc.sync.dma_start(
                        out=dst.rearrange("m (b d) h -> d (m b) h", b=NC2),
                        in_=ost.rearrange("p (q h) -> p q h", h=H),
                    )
    _split_multi_waits(nc)
    return nc


def _split_multi_waits(nc):
    """This walrus build accepts at most one semaphore wait per
    instruction; Tile emits up to ~2-4.  Hoist extra waits onto NoOp
    instructions inserted just before, on the same engine."""
    import concourse.mybir as mybir

    n_split = 0
    for fn in nc.m.functions:
        for bb in fn.blocks:
            insts = bb.instructions
            new = []
            for inst in insts:
                si = inst.sync_info
                waits = list(si.on_wait) if si is not None else []
                if len(waits) > 1:
                    for w in waits[:-1]:
                        nop = mybir.InstNoOp(
                            name=nc.get_next_instruction_name(), ins=[], outs=[]
                        )
                        nop.engine = inst.engine
                        nop.sync_info = mybir.SyncInfo(
                            on_update=[], on_wait=[w]
                        )
                        new.append(nop)
                        n_split += 1
                    si.on_wait = [waits[-1]]
                new.append(inst)
            if n_split:
                bb.instructions = new
    return n_split


def _get_nc(with_bias=False):
    key = ("nc", bool(with_bias))
    if key not in _CACHE:
        _CACHE[key] = _build_nc(with_bias=with_bias)
    return _CACHE[key]


def _prep(G, W, b):
    # Block-diagonal W for phase 3: rows indexed (l, r) with r = m-within-
    # group, cols (r'', h); nonzero only when r == r''.
    MG = 4
    Wbd = np.zeros((K * K, P, MG * H), dtype=np.float32)
    for k in range(K):
        for j in range(K):
            blk = W[k * (K * L) + j * L:k * (K * L) + (j + 1) * L, :]  # [L, H]
            for l in range(L):
                for r in range(MG):
                    Wbd[k * K + j, l * MG + r, r * H:(r + 1) * H] = blk[l]
    Bb = np.tile(b, MG)[None, :].astype(np.float32)
    return np.ascontiguousarray(Wbd), Bb


def kernel(X, G, W, b):
    import ml_dtypes
    from concourse.bass_utils import run_bass_kernel_spmd

    X = np.ascontiguousarray(X, dtype=np.float32)
    G = np.ascontiguousarray(G, dtype=np.float32)
    W = np.ascontiguousarray(W, dtype=np.float32)
    b = np.ascontiguousarray(b, dtype=np.float32)
    nc = _get_nc(with_bias=bool(np.any(b != 0.0)))
    Wr, Bb = _prep(G, W, b)
    Xb = X.astype(ml_dtypes.bfloat16)
    Gb = G.astype(ml_dtypes.bfloat16)
    ones = np.ones((1, P), dtype=np.float32)
    in_maps = [
        {"X": Xb[i], "GB": Gb, "WR": Wr, "BB": Bb, "ONES": ones}
        for i in range(B)
    ]
    res = run_bass_kernel_spmd(nc, in_maps, list(range(B)))
    out = np.stack([res.results[i]["OUT"] for i in range(B)], axis=0)
    return out

